# revision 8
# baseline (speedup 1.0000x reference)
"""Joseph 3D projector on 8 TRN2 NeuronCores — banded-matmul version.

Formulation: for each angle a, out[u, v] = DT * sum_p M_a[p, u] * volT[p, v]
where p = y*128 + x and volT[p, v] = vol[0,0,x,y,v] (the reference's
z-interpolation is an exact identity for this geometry). M_a is ~1.5% dense:
for a fixed contraction slab (a y-line or x-line of the volume) the nonzero
u's lie in a narrow window of width ~128*min(|sin|,|cos|)+4. So instead of
streaming dense M (4MB/angle bf16, DMA-bound at ~212us), we stream only the
per-slab windows in fp8-e3m4 and issue one small matmul per (slab, angle)
accumulating at the window's column offset in PSUM (per-element has_written
semantics: first-touch overwrites, later touches accumulate).

Per angle the contraction axis is chosen as the driving axis (contract over
x with y-slabs when |sin|<=|cos|, else over y with x-slabs), against one of
two SBUF-resident volume layouts (volA=[x,(y,v)], volB=[y,(x,v)], bf16).

Sharding: 15 angle-slots; slot j on core c holds angle k = 8j + c, so all 8
cores share one SPMD program whose per-slot window geometry (width, per-slab
column offsets) covers the slot's 8 consecutive angles.

The stationary operand (the vol slab) is shared by all of a slab's matmuls;
redundant LDWEIGHTS are elided via InstMatmult.ldweights=False.
"""
import numpy as np
import ml_dtypes

D = H = W = 128
V = U = 128
A = 120
S = 128
NCORES = 8
NSLOT = A // NCORES  # 15
T = 0.5 * float(np.sqrt(((W - 1) * 1.0) ** 2 + ((H - 1) * 1.0) ** 2))
DT = 2.0 * T / S

ELIDE_LDW = False      # InstMatmult.ldweights=False proved inert on HW
MGROUP = 16            # slabs per M dma group
PREFETCH = 2           # dma groups emitted ahead of compute


def _build_M(cos_t, sin_t):
    """Dense M[p=(y*W+x), u] float32 for one angle."""
    u_phys = np.arange(U, dtype=np.float64) - (U - 1) / 2.0
    t = -T + (np.arange(S, dtype=np.float64) + 0.5) * DT
    x_idx = (-u_phys[None, :] * sin_t + t[:, None] * cos_t) + (W - 1) / 2.0
    y_idx = (u_phys[None, :] * cos_t + t[:, None] * sin_t) + (H - 1) / 2.0
    x0 = np.floor(x_idx).astype(np.int64)
    y0 = np.floor(y_idx).astype(np.int64)
    wx = x_idx - x0
    wy = y_idx - y0
    Mflat = np.zeros(H * W * U, np.float32)
    uu = np.broadcast_to(np.arange(U, dtype=np.int64)[None, :], (S, U))
    for dy, dx in ((0, 0), (0, 1), (1, 0), (1, 1)):
        yi = y0 + dy
        xi = x0 + dx
        w = (wy if dy else 1 - wy) * (wx if dx else 1 - wx)
        valid = (xi >= 0) & (xi <= W - 1) & (yi >= 0) & (yi <= H - 1)
        p = np.clip(yi, 0, H - 1) * W + np.clip(xi, 0, W - 1)
        flat = (p * U + uu)[valid]
        Mflat += np.bincount(flat, weights=w[valid].astype(np.float64),
                             minlength=H * W * U).astype(np.float32)
    return Mflat.reshape(H * W, U)


def _schedule(angles):
    """Slot geometry shared by all cores: per slot j (angles 8j..8j+7):
    contraction axis, window width, per-slab window offsets."""
    Ms = np.stack([_build_M(np.cos(np.float64(a)), np.sin(np.float64(a)))
                   for a in angles])              # [A, H*W, U]
    axes, widths, offs = [], [], []
    for j in range(NSLOT):
        ks = list(range(NCORES * j, NCORES * j + NCORES))
        th = [float(angles[k]) for k in ks]
        s_m = np.mean([abs(np.sin(t)) for t in th])
        c_m = np.mean([abs(np.cos(t)) for t in th])
        ax = 0 if s_m <= c_m else 1
        lo = np.full(S, U, np.int64)
        hi = np.full(S, -1, np.int64)
        for k in ks:
            Mr = Ms[k].reshape(H, W, U)
            sl = Mr if ax == 0 else Mr.transpose(1, 0, 2)  # [slab, kdim, u]
            nz = sl.any(axis=1)                            # [slab, u]
            any_s = nz.any(axis=1)
            first = nz.argmax(axis=1)
            last = U - 1 - nz[:, ::-1].argmax(axis=1)
            lo = np.where(any_s, np.minimum(lo, first), lo)
            hi = np.where(any_s, np.maximum(hi, last), hi)
        w = int((hi - lo).max() + 1)
        assert 0 < w <= U, w
        off = np.minimum(np.maximum(lo, 0), U - w)
        off = np.where(hi < 0, 0, off).astype(np.int64)
        # every psum column of the slot's 128-wide region must be written
        # by at least one slab matmul (unwritten psum is garbage)
        cover = np.zeros(U, bool)
        for s in range(S):
            cover[off[s]:off[s] + w] = True
        assert cover.all(), f"slot {j}: uncovered psum cols"
        axes.append(ax)
        widths.append(w)
        offs.append(off)
    return Ms, np.array(axes), np.array(widths), np.stack(offs)


def _pack_core(Ms, axes, widths, offs, cum0, cum1, W0, W1, core):
    """Banded fp8 M streams for one core: m0 [128, 128*W0], m1 [128, 128*W1]."""
    f8 = ml_dtypes.float8_e3m4
    m0 = np.zeros((S, W0 * S), f8)
    m1 = np.zeros((S, W1 * S), f8)
    tt = np.arange(U)
    for j in range(NSLOT):
        k = NCORES * j + core
        Mr = Ms[k].reshape(H, W, U)
        w, off = int(widths[j]), offs[j]
        idx = off[:, None] + tt[None, :w]                  # [slab, w]
        if axes[j] == 0:
            band = np.take_along_axis(Mr, idx[:, None, :], axis=2)   # [y, x, w]
            dst = m0.reshape(S, S, W0)                     # [x, y, W0]
            dst[:, :, cum0[j]:cum0[j] + w] = band.transpose(1, 0, 2).astype(f8)
        else:
            band = np.take_along_axis(Mr.transpose(1, 0, 2), idx[:, None, :],
                                      axis=2)              # [x, y, w]
            dst = m1.reshape(S, S, W1)                     # [y, x, W1]
            dst[:, :, cum1[j]:cum1[j] + w] = band.transpose(1, 0, 2).astype(f8)
    return m0, m1


_COMPILED = {}


def _get_compiled(angles):
    key = hash(angles.tobytes())
    if key in _COMPILED:
        return _COMPILED[key]
    from contextlib import ExitStack
    import concourse.bacc as bacc
    import concourse.tile as tile
    import concourse.mybir as mybir

    Ms, axes, widths, offs = _schedule(angles)
    axis0_slots = [j for j in range(NSLOT) if axes[j] == 0]
    axis1_slots = [j for j in range(NSLOT) if axes[j] == 1]
    order = axis0_slots + axis1_slots        # psum/out column order
    pos = {j: p for p, j in enumerate(order)}
    banks = [order[b * 4:b * 4 + 4] for b in range(4)]
    bank_of = {j: b for b, bs in enumerate(banks) for j in bs}
    block_of = {j: bs.index(j) for bs in banks for j in bs}
    cum0 = {}
    c = 0
    for j in axis0_slots:
        cum0[j] = c
        c += int(widths[j])
    W0 = c
    cum1 = {}
    c = 0
    for j in axis1_slots:
        cum1[j] = c
        c += int(widths[j])
    W1 = c

    nc = bacc.Bacc("TRN2", target_bir_lowering=False, debug=False,
                   enable_asserts=False, num_devices=NCORES)
    bf16 = mybir.dt.bfloat16
    f8 = mybir.dt.float8e3
    f32 = mybir.dt.float32

    volA_d = nc.dram_tensor("volA", [S, H * D], bf16, kind="ExternalInput").ap()
    volB_d = nc.dram_tensor("volB", [S, H * D], bf16, kind="ExternalInput").ap()
    m0_d = nc.dram_tensor("m0", [S, S * W0], f8, kind="ExternalInput").ap()
    m1_d = nc.dram_tensor("m1", [S, S * W1], f8, kind="ExternalInput").ap()
    out_d = nc.dram_tensor("out", [V, NSLOT * U], f32, kind="ExternalOutput").ap()

    NG = S // MGROUP  # dma groups per axis

    with tile.TileContext(nc) as tc:
        with ExitStack() as ctx:
            sbuf = ctx.enter_context(tc.tile_pool(name="sbuf", bufs=1))
            m0pool = ctx.enter_context(tc.tile_pool(name="m0", bufs=PREFETCH + 2))
            m1pool = ctx.enter_context(tc.tile_pool(name="m1", bufs=PREFETCH + 2))
            psum = ctx.enter_context(tc.tile_pool(name="psum", bufs=1, space="PSUM"))

            volA_sb = sbuf.tile([S, H * D], bf16)
            volB_sb = sbuf.tile([S, H * D], bf16)
            out_sb = sbuf.tile([V, NSLOT * U], f32)
            ps = [psum.tile([V, 512], f32, name=f"ps{b}") for b in range(4)]
            m0_t, m1_t = {}, {}

            # spread the streams over the available DMA issuers: the two
            # hardware DGE queues (sync=SP, scalar=Activation) carry the M
            # bands; the gpsimd SWDGE queue carries the volume layouts.
            def issue_group(g):
                if g >= NG:
                    return
                cs = MGROUP * H  # vol cols per group
                nc.gpsimd.dma_start(
                    volA_sb[:, g * cs:(g + 1) * cs], volA_d[:, g * cs:(g + 1) * cs])
                nc.gpsimd.dma_start(
                    volB_sb[:, g * cs:(g + 1) * cs], volB_d[:, g * cs:(g + 1) * cs])
                t0 = m0pool.tile([S, MGROUP * W0], f8, tag="m0")
                nc.sync.dma_start(
                    t0[:], m0_d[:, g * MGROUP * W0:(g + 1) * MGROUP * W0])
                m0_t[g] = t0
                t1 = m1pool.tile([S, MGROUP * W1], f8, tag="m1")
                nc.scalar.dma_start(
                    t1[:], m1_d[:, g * MGROUP * W1:(g + 1) * MGROUP * W1])
                m1_t[g] = t1

            for g in range(PREFETCH + 1):
                issue_group(g)

            # program-order first/last matmul per psum bank, for start/stop
            first_of_bank = {}
            last_of_bank = {}
            seq = []
            for s in range(S):
                for j in axis0_slots + axis1_slots:
                    seq.append((s, j))
            for i, (s, j) in enumerate(seq):
                b = bank_of[j]
                if b not in first_of_bank:
                    first_of_bank[b] = i
                last_of_bank[b] = i

            i = 0
            for s in range(S):
                g = s // MGROUP
                if s % MGROUP == 0:
                    issue_group(g + PREFETCH + 1)
                for phase, slots in ((0, axis0_slots), (1, axis1_slots)):
                    vol_sb = volA_sb if phase == 0 else volB_sb
                    mt = m0_t[g] if phase == 0 else m1_t[g]
                    Wp = W0 if phase == 0 else W1
                    cump = cum0 if phase == 0 else cum1
                    lhsT = vol_sb[:, s * D:(s + 1) * D]
                    for idx, j in enumerate(slots):
                        w = int(widths[j])
                        b = bank_of[j]
                        col = block_of[j] * U + int(offs[j][s])
                        mm = nc.tensor.matmul(
                            ps[b][:, col:col + w],
                            lhsT=lhsT,
                            rhs=mt[:, (s % MGROUP) * Wp + cump[j]:
                                   (s % MGROUP) * Wp + cump[j] + w],
                            start=(i == first_of_bank[b]),
                            stop=(i == last_of_bank[b]),
                        )
                        if ELIDE_LDW and idx > 0:
                            mm.ins.ldweights = False
                        i += 1

            base = 0
            for b in range(4):
                n = len(banks[b]) * U
                nc.scalar.mul(out_sb[:, base:base + n], ps[b][:, 0:n], float(DT))
                base += n
            nc.sync.dma_start(out_d, out_sb[:])

    nc.compile()
    meta = dict(nc=nc, Ms=Ms, axes=axes, widths=widths, offs=offs,
                cum0=cum0, cum1=cum1, W0=W0, W1=W1, order=order)
    _COMPILED[key] = meta
    return meta


def kernel(vol, angles):
    from concourse.bass_utils import run_bass_kernel_spmd

    vol = np.asarray(vol, dtype=np.float32)
    angles = np.asarray(angles, dtype=np.float32)
    meta = _get_compiled(angles)
    nc = meta["nc"]

    volA = vol[0, 0].reshape(S, H * D).astype(ml_dtypes.bfloat16)
    volB = np.ascontiguousarray(vol[0, 0].transpose(1, 0, 2)).reshape(
        S, H * D).astype(ml_dtypes.bfloat16)
    in_maps = []
    for c in range(NCORES):
        m0, m1 = _pack_core(meta["Ms"], meta["axes"], meta["widths"],
                            meta["offs"], meta["cum0"], meta["cum1"],
                            meta["W0"], meta["W1"], c)
        in_maps.append({"volA": volA, "volB": volB, "m0": m0, "m1": m1})

    res = run_bass_kernel_spmd(nc, in_maps, core_ids=list(range(NCORES)))
    global _LAST_RES
    _LAST_RES = res
    full = np.empty((1, 1, U, A, V), np.float32)
    for c, r in enumerate(res.results):
        rc = r["out"]                       # [v, pos*128 + u]
        for p, j in enumerate(meta["order"]):
            k = NCORES * j + c
            full[0, 0, :, k, :] = rc[:, p * U:(p + 1) * U].T
    return full


# revision 9
# speedup vs baseline: 1.0951x; 1.0951x over previous
"""Joseph 3D projector on 8 TRN2 NeuronCores — banded-matmul version.

Formulation: for each angle a, out[u, v] = DT * sum_p M_a[p, u] * volT[p, v]
where p = y*128 + x and volT[p, v] = vol[0,0,x,y,v] (the reference's
z-interpolation is an exact identity for this geometry). M_a is ~1.5% dense:
for a fixed contraction slab (a y-line or x-line of the volume) the nonzero
u's lie in a narrow window of width ~128*min(|sin|,|cos|)+4. So instead of
streaming dense M (4MB/angle bf16, DMA-bound at ~212us), we stream only the
per-slab windows in fp8-e3m4 and issue one small matmul per (slab, angle)
accumulating at the window's column offset in PSUM (per-element has_written
semantics: first-touch overwrites, later touches accumulate).

Per angle the contraction axis is chosen as the driving axis (contract over
x with y-slabs when |sin|<=|cos|, else over y with x-slabs), against one of
two SBUF-resident volume layouts (volA=[x,(y,v)], volB=[y,(x,v)], bf16).

Sharding: 15 angle-slots; slot j on core c holds angle k = 8j + c, so all 8
cores share one SPMD program whose per-slot window geometry (width, per-slab
column offsets) covers the slot's 8 consecutive angles.

The stationary operand (the vol slab) is shared by all of a slab's matmuls;
redundant LDWEIGHTS are elided via InstMatmult.ldweights=False.
"""
import numpy as np
import ml_dtypes

D = H = W = 128
V = U = 128
A = 120
S = 128
NCORES = 8
NSLOT = A // NCORES  # 15
T = 0.5 * float(np.sqrt(((W - 1) * 1.0) ** 2 + ((H - 1) * 1.0) ** 2))
DT = 2.0 * T / S

ELIDE_LDW = False      # InstMatmult.ldweights=False proved inert on HW
MGROUP = 16            # slabs per M dma group
PREFETCH = 2           # dma groups emitted ahead of compute


def _build_M(cos_t, sin_t):
    """Dense M[p=(y*W+x), u] float32 for one angle."""
    u_phys = np.arange(U, dtype=np.float64) - (U - 1) / 2.0
    t = -T + (np.arange(S, dtype=np.float64) + 0.5) * DT
    x_idx = (-u_phys[None, :] * sin_t + t[:, None] * cos_t) + (W - 1) / 2.0
    y_idx = (u_phys[None, :] * cos_t + t[:, None] * sin_t) + (H - 1) / 2.0
    x0 = np.floor(x_idx).astype(np.int64)
    y0 = np.floor(y_idx).astype(np.int64)
    wx = x_idx - x0
    wy = y_idx - y0
    Mflat = np.zeros(H * W * U, np.float32)
    uu = np.broadcast_to(np.arange(U, dtype=np.int64)[None, :], (S, U))
    for dy, dx in ((0, 0), (0, 1), (1, 0), (1, 1)):
        yi = y0 + dy
        xi = x0 + dx
        w = (wy if dy else 1 - wy) * (wx if dx else 1 - wx)
        valid = (xi >= 0) & (xi <= W - 1) & (yi >= 0) & (yi <= H - 1)
        p = np.clip(yi, 0, H - 1) * W + np.clip(xi, 0, W - 1)
        flat = (p * U + uu)[valid]
        Mflat += np.bincount(flat, weights=w[valid].astype(np.float64),
                             minlength=H * W * U).astype(np.float32)
    return Mflat.reshape(H * W, U)


def _schedule(angles):
    """Slot geometry shared by all cores: per slot j (angles 8j..8j+7):
    contraction axis, window width, per-slab window offsets."""
    Ms = np.stack([_build_M(np.cos(np.float64(a)), np.sin(np.float64(a)))
                   for a in angles])              # [A, H*W, U]
    axes, widths, offs = [], [], []
    for j in range(NSLOT):
        ks = list(range(NCORES * j, NCORES * j + NCORES))
        th = [float(angles[k]) for k in ks]
        s_m = np.mean([abs(np.sin(t)) for t in th])
        c_m = np.mean([abs(np.cos(t)) for t in th])
        ax = 0 if s_m <= c_m else 1
        lo = np.full(S, U, np.int64)
        hi = np.full(S, -1, np.int64)
        for k in ks:
            Mr = Ms[k].reshape(H, W, U)
            sl = Mr if ax == 0 else Mr.transpose(1, 0, 2)  # [slab, kdim, u]
            nz = sl.any(axis=1)                            # [slab, u]
            any_s = nz.any(axis=1)
            first = nz.argmax(axis=1)
            last = U - 1 - nz[:, ::-1].argmax(axis=1)
            lo = np.where(any_s, np.minimum(lo, first), lo)
            hi = np.where(any_s, np.maximum(hi, last), hi)
        w = int((hi - lo).max() + 1)
        assert 0 < w <= U, w
        off = np.minimum(np.maximum(lo, 0), U - w)
        off = np.where(hi < 0, 0, off).astype(np.int64)
        # every psum column of the slot's 128-wide region must be written
        # by at least one slab matmul (unwritten psum is garbage)
        cover = np.zeros(U, bool)
        for s in range(S):
            cover[off[s]:off[s] + w] = True
        assert cover.all(), f"slot {j}: uncovered psum cols"
        axes.append(ax)
        widths.append(w)
        offs.append(off)
    return Ms, np.array(axes), np.array(widths), np.stack(offs)


def _pack_core(Ms, axes, widths, offs, cum0, cum1, W0, W1, core):
    """Banded fp8 M streams for one core: m0 [128, 128*W0], m1 [128, 128*W1]."""
    f8 = ml_dtypes.float8_e3m4
    m0 = np.zeros((S, W0 * S), f8)
    m1 = np.zeros((S, W1 * S), f8)
    tt = np.arange(U)
    for j in range(NSLOT):
        k = NCORES * j + core
        Mr = Ms[k].reshape(H, W, U)
        w, off = int(widths[j]), offs[j]
        idx = off[:, None] + tt[None, :w]                  # [slab, w]
        if axes[j] == 0:
            band = np.take_along_axis(Mr, idx[:, None, :], axis=2)   # [y, x, w]
            dst = m0.reshape(S, S, W0)                     # [x, y, W0]
            dst[:, :, cum0[j]:cum0[j] + w] = band.transpose(1, 0, 2).astype(f8)
        else:
            band = np.take_along_axis(Mr.transpose(1, 0, 2), idx[:, None, :],
                                      axis=2)              # [x, y, w]
            dst = m1.reshape(S, S, W1)                     # [y, x, W1]
            dst[:, :, cum1[j]:cum1[j] + w] = band.transpose(1, 0, 2).astype(f8)
    return m0, m1


_COMPILED = {}


def _get_compiled(angles):
    key = hash(angles.tobytes())
    if key in _COMPILED:
        return _COMPILED[key]
    from contextlib import ExitStack
    import concourse.bacc as bacc
    import concourse.tile as tile
    import concourse.mybir as mybir

    Ms, axes, widths, offs = _schedule(angles)
    axis0_slots = [j for j in range(NSLOT) if axes[j] == 0]
    axis1_slots = [j for j in range(NSLOT) if axes[j] == 1]
    order = axis0_slots + axis1_slots        # psum/out column order
    pos = {j: p for p, j in enumerate(order)}
    banks = [order[b * 4:b * 4 + 4] for b in range(4)]
    bank_of = {j: b for b, bs in enumerate(banks) for j in bs}
    block_of = {j: bs.index(j) for bs in banks for j in bs}
    cum0 = {}
    c = 0
    for j in axis0_slots:
        cum0[j] = c
        c += int(widths[j])
    W0 = c
    cum1 = {}
    c = 0
    for j in axis1_slots:
        cum1[j] = c
        c += int(widths[j])
    W1 = c

    nc = bacc.Bacc("TRN2", target_bir_lowering=False, debug=False,
                   enable_asserts=False, num_devices=NCORES)
    bf16 = mybir.dt.bfloat16
    f8 = mybir.dt.float8e3
    f32 = mybir.dt.float32

    volA_d = nc.dram_tensor("volA", [S, H * D], bf16, kind="ExternalInput").ap()
    volB_d = nc.dram_tensor("volB", [S, H * D], bf16, kind="ExternalInput").ap()
    m0_d = nc.dram_tensor("m0", [S, S * W0], f8, kind="ExternalInput").ap()
    m1_d = nc.dram_tensor("m1", [S, S * W1], f8, kind="ExternalInput").ap()
    out_d = nc.dram_tensor("out", [V, NSLOT * U], f32, kind="ExternalOutput").ap()

    NG = S // MGROUP  # dma groups per axis

    with tile.TileContext(nc) as tc:
        with ExitStack() as ctx:
            sbuf = ctx.enter_context(tc.tile_pool(name="sbuf", bufs=1))
            psum = ctx.enter_context(tc.tile_pool(name="psum", bufs=1, space="PSUM"))

            volA_sb = sbuf.tile([S, H * D], bf16)
            volB_sb = sbuf.tile([S, H * D], bf16)
            m0_sb = sbuf.tile([S, S * W0], f8)
            m1_sb = sbuf.tile([S, S * W1], f8)
            out_sb = sbuf.tile([V, NSLOT * U], f32)
            ps = [psum.tile([V, 512], f32, name=f"ps{b}") for b in range(4)]

            # Everything is SBUF-resident; chunked DMAs give fine-grained
            # deps so compute chases the stream frontier. Queue plan: the
            # sync HW queue carries phase-0's stream (m0) then volB; the
            # scalar HW queue pre-stocks volA then phase-1's m1.
            cs = MGROUP * H  # vol cols per chunk
            nc.scalar.dma_start(volA_sb[:, 0:cs], volA_d[:, 0:cs])
            for g in range(NG):
                a, b_ = g * MGROUP * W0, (g + 1) * MGROUP * W0
                nc.sync.dma_start(m0_sb[:, a:b_], m0_d[:, a:b_])
            for g in range(1, NG):
                nc.scalar.dma_start(
                    volA_sb[:, g * cs:(g + 1) * cs], volA_d[:, g * cs:(g + 1) * cs])
            for g in range(NG):
                a, b_ = g * MGROUP * W1, (g + 1) * MGROUP * W1
                nc.scalar.dma_start(m1_sb[:, a:b_], m1_d[:, a:b_])
            for g in range(NG):
                nc.sync.dma_start(
                    volB_sb[:, g * cs:(g + 1) * cs], volB_d[:, g * cs:(g + 1) * cs])

            # program-order first/last matmul per psum bank, for start/stop
            first_of_bank = {}
            last_of_bank = {}
            seq = []
            for phase, slots in ((0, axis0_slots), (1, axis1_slots)):
                for s in range(S):
                    for j in slots:
                        seq.append(j)
            for i, j in enumerate(seq):
                b = bank_of[j]
                if b not in first_of_bank:
                    first_of_bank[b] = i
                last_of_bank[b] = i

            phase_banks = [sorted({bank_of[j] for j in axis0_slots}),
                           sorted({bank_of[j] for j in axis1_slots})]
            out_base = {}
            base = 0
            for b in range(4):
                out_base[b] = base
                base += len(banks[b]) * U

            i = 0
            for phase, slots in ((0, axis0_slots), (1, axis1_slots)):
                vol_sb = volA_sb if phase == 0 else volB_sb
                m_sb = m0_sb if phase == 0 else m1_sb
                Wp = W0 if phase == 0 else W1
                cump = cum0 if phase == 0 else cum1
                for s in range(S):
                    lhsT = vol_sb[:, s * D:(s + 1) * D]
                    for j in slots:
                        w = int(widths[j])
                        b = bank_of[j]
                        col = block_of[j] * U + int(offs[j][s])
                        nc.tensor.matmul(
                            ps[b][:, col:col + w],
                            lhsT=lhsT,
                            rhs=m_sb[:, s * Wp + cump[j]:s * Wp + cump[j] + w],
                            start=(i == first_of_bank[b]),
                            stop=(i == last_of_bank[b]),
                        )
                        i += 1
                # flush this phase's banks while the next phase computes
                for b in phase_banks[phase]:
                    n = len(banks[b]) * U
                    nc.scalar.mul(out_sb[:, out_base[b]:out_base[b] + n],
                                  ps[b][:, 0:n], float(DT))
                    nc.sync.dma_start(out_d[:, out_base[b]:out_base[b] + n],
                                      out_sb[:, out_base[b]:out_base[b] + n])

    nc.compile()
    meta = dict(nc=nc, Ms=Ms, axes=axes, widths=widths, offs=offs,
                cum0=cum0, cum1=cum1, W0=W0, W1=W1, order=order)
    _COMPILED[key] = meta
    return meta


def kernel(vol, angles):
    from concourse.bass_utils import run_bass_kernel_spmd

    vol = np.asarray(vol, dtype=np.float32)
    angles = np.asarray(angles, dtype=np.float32)
    meta = _get_compiled(angles)
    nc = meta["nc"]

    volA = vol[0, 0].reshape(S, H * D).astype(ml_dtypes.bfloat16)
    volB = np.ascontiguousarray(vol[0, 0].transpose(1, 0, 2)).reshape(
        S, H * D).astype(ml_dtypes.bfloat16)
    in_maps = []
    for c in range(NCORES):
        m0, m1 = _pack_core(meta["Ms"], meta["axes"], meta["widths"],
                            meta["offs"], meta["cum0"], meta["cum1"],
                            meta["W0"], meta["W1"], c)
        in_maps.append({"volA": volA, "volB": volB, "m0": m0, "m1": m1})

    res = run_bass_kernel_spmd(nc, in_maps, core_ids=list(range(NCORES)))
    global _LAST_RES
    _LAST_RES = res
    full = np.empty((1, 1, U, A, V), np.float32)
    for c, r in enumerate(res.results):
        rc = r["out"]                       # [v, pos*128 + u]
        for p, j in enumerate(meta["order"]):
            k = NCORES * j + c
            full[0, 0, :, k, :] = rc[:, p * U:(p + 1) * U].T
    return full
